# revision 13
# baseline (speedup 1.0000x reference)
"""DeepFM backbone on 8 TRN2 NeuronCores — v4.

Sharding: batch 16384 -> 2048 per core; packed table replicated.
Gather strategy per core:
  - fields 0-2 (vocab 1M/100K/100K): per-(field,tile) indirect DMA (48)
  - fields 3-5 (vocab 10K): ONE dma_gather, 6144 idxs, 256B padded rows
  - fields 6-9 (vocab 1K):  ONE dma_gather, 8192 idxs, 256B padded rows
  - fields 10-19 (560 rows total): one-hot x block-diagonal matmul on PE
Per 128-sample tile: unified hsp [P,20,17] bf16 -> hcomp field-major,
FM1/FM2 pieces on DVE/ACT, 5 PE transposes -> hT, drains alternate
ACT/DVE. DNN in bf16 (f32 PSUM): L1/L2 with bn_stats/bn_aggr, two tiny
AllReduces for cross-core BN, L3 + FM add, f32 output.
"""
import sys
sys.path.insert(0, '/opt/trn_rl_repo')
import numpy as np
import ml_dtypes

import concourse.bass as bass
import concourse.bacc as bacc
import concourse.tile as tile
import concourse.mybir as mybir
from concourse.bass_utils import run_bass_kernel_spmd

# ---- problem constants (hardcoded per contract) ----
SPARSE_DIMS = [1000000, 100000, 100000, 10000, 10000, 10000, 1000, 1000, 1000,
               1000, 100, 100, 100, 100, 50, 50, 20, 20, 10, 10]
NS = 20
ND = 13
NF = NS + ND                 # 33 fields
FEAT = 16
H1, H2 = 256, 128
B = 16384
N_CORES = 8
BC = B // N_CORES            # 2048 samples per core
P = 128
NT = BC // P                 # 16 batch tiles per core
R_TOTAL = int(np.sum(SPARSE_DIMS))
OFFSETS = np.concatenate([[0], np.cumsum(SPARSE_DIMS)[:-1]]).astype(np.int64)
BN_EPS = 1e-5
F32 = mybir.dt.float32
BF16 = mybir.dt.bfloat16
I32 = mybir.dt.int32
I16 = mybir.dt.int16
AF = mybir.ActivationFunctionType
ALU = mybir.AluOpType
BF = ml_dtypes.bfloat16

E = FEAT + 1                 # 17 table cols (emb + fm1)
EW = 128                     # padded row width for dma_gather (256B)
NIND = 3                     # fields via indirect DMA
R35 = 30000                  # rows fields 3-5
R69 = 4000                   # rows fields 6-9
N35 = NT * P * 3             # 6144 idxs
N69 = NT * P * 4             # 8192 idxs
SMALL = list(range(10, 20))  # fields via one-hot matmul
SVOC = [SPARSE_DIMS[f] for f in SMALL]
SOFF = np.concatenate([[0], np.cumsum(SVOC)[:-1]]).astype(np.int64)
SROWS = int(np.sum(SVOC))    # 560
NCH = 5                      # one-hot row chunks of 128
# W1 row chunks over the 528-dim field-major input
CB = [0, 128, 256, 384, 512, 528]
NK = 5

_CACHE = {}


def _build(reps=1, skip_gather=False, skip_cc=False):
    nc = bacc.Bacc("TRN2", target_bir_lowering=False, debug=False,
                   num_devices=N_CORES)
    # ---- DRAM I/O ----
    tbl = nc.dram_tensor("tbl", [R_TOTAL, E], BF16, kind="ExternalInput")
    t35_d = nc.dram_tensor("t35_d", [R35, EW], BF16, kind="ExternalInput")
    t69_d = nc.dram_tensor("t69_d", [R69, EW], BF16, kind="ExternalInput")
    gid_d = nc.dram_tensor("gid_d", [P, NT, NIND], I32, kind="ExternalInput")
    i35_d = nc.dram_tensor("i35_d", [P, N35 // 16], I16, kind="ExternalInput")
    i69_d = nc.dram_tensor("i69_d", [P, N69 // 16], I16, kind="ExternalInput")
    xde_d = nc.dram_tensor("xde_d", [P, NT, ND], BF16, kind="ExternalInput")
    xdeT_d = nc.dram_tensor("xdeT_d", [ND, BC], BF16, kind="ExternalInput")
    dfw_d = nc.dram_tensor("dfw_d", [ND, 1], BF16, kind="ExternalInput")
    oh_d = nc.dram_tensor("oh_d", [P, NT, NCH, P], BF16, kind="ExternalInput")
    bd_d = nc.dram_tensor("bd_d", [P, NCH, 10 * E], BF16, kind="ExternalInput")
    w1_d = nc.dram_tensor("w1_d", [P, NK, H1], BF16, kind="ExternalInput")
    w2_d = nc.dram_tensor("w2_d", [P, 2, H2], BF16, kind="ExternalInput")
    w3_d = nc.dram_tensor("w3_d", [P, 1], BF16, kind="ExternalInput")
    vec1_d = nc.dram_tensor("vec1_d", [P, 8], F32, kind="ExternalInput")
    vec2_d = nc.dram_tensor("vec2_d", [P, 4], F32, kind="ExternalInput")
    dvec_d = nc.dram_tensor("dvec_d", [P, 2 * ND * FEAT + ND], BF16,
                            kind="ExternalInput")
    id_d = nc.dram_tensor("id_d", [P, P], BF16, kind="ExternalInput")
    out = nc.dram_tensor("out", [P, reps, NT], F32, kind="ExternalOutput")

    with tile.TileContext(nc) as tc:
        with tc.tile_pool(name="const", bufs=1) as cp, \
             tc.tile_pool(name="hsp", bufs=3) as hp, \
             tc.tile_pool(name="scr", bufs=3) as sp, \
             tc.tile_pool(name="big", bufs=1) as bigp, \
             tc.tile_pool(name="gat", bufs=2) as gp, \
             tc.tile_pool(name="ps_t", bufs=2, space="PSUM") as ps_t, \
             tc.tile_pool(name="ps_g", bufs=2, space="PSUM") as ps_g, \
             tc.tile_pool(name="ps_z", bufs=2, space="PSUM") as ps_z, \
             tc.tile_pool(name="ps_s", bufs=1, space="PSUM") as ps_s, \
             tc.tile_pool(name="dram", bufs=2, space="DRAM") as dp:

            # ---- load constants ----
            gid_t = cp.tile([P, NT, NIND], I32)
            nc.sync.dma_start(out=gid_t[:], in_=gid_d[:])
            i35_t = cp.tile([P, N35 // 16], I16)
            nc.sync.dma_start(out=i35_t[:], in_=i35_d[:])
            i69_t = cp.tile([P, N69 // 16], I16)
            nc.sync.dma_start(out=i69_t[:], in_=i69_d[:])
            xde_t = cp.tile([P, NT, ND], BF16)
            nc.sync.dma_start(out=xde_t[:], in_=xde_d[:])
            xdeT_t = cp.tile([ND, BC], BF16)
            nc.sync.dma_start(out=xdeT_t[:], in_=xdeT_d[:])
            dfw_t = cp.tile([ND, 1], BF16)
            nc.sync.dma_start(out=dfw_t[:], in_=dfw_d[:])
            oh_t = cp.tile([P, NT, NCH, P], BF16)
            nc.sync.dma_start(out=oh_t[:], in_=oh_d[:])
            bd_t = cp.tile([P, NCH, 10 * E], BF16)
            nc.sync.dma_start(out=bd_t[:], in_=bd_d[:])
            w1_t = cp.tile([P, NK, H1], BF16)
            nc.sync.dma_start(out=w1_t[:], in_=w1_d[:])
            w2_t = cp.tile([P, 2, H2], BF16)
            nc.sync.dma_start(out=w2_t[:], in_=w2_d[:])
            w3_t = cp.tile([P, 1], BF16)
            nc.sync.dma_start(out=w3_t[:], in_=w3_d[:])
            vec1_t = cp.tile([P, 8], F32)
            nc.sync.dma_start(out=vec1_t[:], in_=vec1_d[:])
            vec2_t = cp.tile([P, 4], F32)
            nc.sync.dma_start(out=vec2_t[:], in_=vec2_d[:])
            dv_t = cp.tile([P, 2 * ND * FEAT + ND], BF16)
            nc.sync.dma_start(out=dv_t[:], in_=dvec_d[:])
            ident = cp.tile([P, P], BF16)
            nc.sync.dma_start(out=ident[:], in_=id_d[:])

            for _rep in range(reps):
                hT = bigp.tile([P, NK, BC], BF16, tag="hT")
                z1T = [bigp.tile([P, BC], BF16, name=f"z1T{m}", tag=f"z1T{m}")
                       for m in range(2)]
                z2T = bigp.tile([P, BC], BF16, tag="z2T")
                g35 = gp.tile([P, NT * 3, EW], BF16, tag="g35")
                g69 = gp.tile([P, NT * 4, EW], BF16, tag="g69")
                fm2sb = bigp.tile([P, NT], F32, tag="fm2sb")
                fmA = bigp.tile([P, NT], F32, tag="fmA")
                st1 = bigp.tile([P, 8, 6], F32, tag="st1")
                st2 = bigp.tile([P, 4, 6], F32, tag="st2")
                out_sb = bigp.tile([P, NT], F32, tag="out_sb")

                cc1_in = dp.tile([P, 4], F32, tag="cc1i")
                cc1_out = dp.tile([P, 4], F32, tag="cc1o")
                cc2_in = dp.tile([P, 2], F32, tag="cc2i")
                cc2_out = dp.tile([P, 2], F32, tag="cc2o")

                # ---- bulk gathers for fields 3-9 ----
                if not skip_gather:
                    nc.gpsimd.dma_gather(
                        out_ap=g35[:], in_ap=t35_d[:], idxs_ap=i35_t[:],
                        num_idxs=N35, num_idxs_reg=N35, elem_size=EW,
                        single_packet=False)
                    nc.gpsimd.dma_gather(
                        out_ap=g69[:], in_ap=t69_d[:], idxs_ap=i69_t[:],
                        num_idxs=N69, num_idxs_reg=N69, elem_size=EW,
                        single_packet=False)
                else:
                    nc.vector.memset(g35[:, :, 0:E], 0.01)
                    nc.vector.memset(g69[:, :, 0:E], 0.01)

                # ================= per-tile loop =================
                for t in range(NT):
                    hsp = hp.tile([P, NS, E], BF16, tag="hsp")
                    # fields 0-2: per-partition indirect gathers
                    if skip_gather:
                        nc.vector.memset(hsp[:, 0:NIND, :], 0.01)
                    else:
                        for f in range(NIND):
                            nc.gpsimd.indirect_dma_start(
                                out=hsp[:, f, :], out_offset=None, in_=tbl[:],
                                in_offset=bass.IndirectOffsetOnAxis(
                                    ap=gid_t[:, t, f:f + 1], axis=0))
                    # fields 3-5 / 6-9 from the bulk gathers
                    s35 = bass.AP(g35.tensor, g35[:, t * 3, 0:1].offset,
                                  [g35[:].ap[0], (EW, 3), (1, E)])
                    nc.vector.tensor_copy(out=hsp[:, 3:6, :], in_=s35)
                    s69 = bass.AP(g69.tensor, g69[:, t * 4, 0:1].offset,
                                  [g69[:].ap[0], (EW, 4), (1, E)])
                    nc.vector.tensor_copy(out=hsp[:, 6:10, :], in_=s69)
                    # fields 10-19: one-hot x block-diagonal table
                    ge = ps_g.tile([P, 10 * E], F32, tag="ge")
                    for c in range(NCH):
                        cr = min(P, SROWS - c * P)
                        nc.tensor.matmul(
                            out=ge[:], lhsT=oh_t[0:cr, t, c, :],
                            rhs=bd_t[0:cr, c, :],
                            start=(c == 0), stop=(c == NCH - 1))
                    gev = bass.AP(hsp.tensor, hsp[:, 10, 0:1].offset,
                                  [hsp[:].ap[0], (1, 10 * E)])
                    nc.scalar.activation(out=gev, in_=ge[:], func=AF.Copy)

                    # hcomp: unified field-major bf16 [P, 33*16]
                    hcomp = sp.tile([P, NF * FEAT], BF16, tag="hcomp")
                    hc_sp = bass.AP(hcomp.tensor, hcomp[:].offset,
                                    [hcomp[:].ap[0], (FEAT, NS), (1, FEAT)])
                    hs_emb = bass.AP(hsp.tensor, hsp[:].offset,
                                     [hsp[:].ap[0], (E, NS), (1, FEAT)])
                    nc.vector.tensor_copy(out=hc_sp, in_=hs_emb)
                    # dense embeddings into hcomp cols 320:528
                    x3 = bass.AP(xde_t.tensor, xde_t[:, t, :].offset,
                                 [xde_t[:].ap[0], (1, ND), (0, FEAT)])
                    dw3 = bass.AP(dv_t.tensor, dv_t[:].offset,
                                  [dv_t[:].ap[0], (FEAT, ND), (1, FEAT)])
                    de = bass.AP(hcomp.tensor, hcomp[:, NS * FEAT:].offset,
                                 [hcomp[:].ap[0], (1, ND * FEAT)])
                    de2 = bass.AP(hcomp.tensor, hcomp[:, NS * FEAT:].offset,
                                  [hcomp[:].ap[0], (FEAT, ND), (1, FEAT)])
                    nc.vector.tensor_tensor(out=de2, in0=x3, in1=dw3,
                                            op=ALU.mult)
                    nc.vector.tensor_tensor(
                        out=de, in0=de, in1=dv_t[:, ND * FEAT:2 * ND * FEAT],
                        op=ALU.add)

                    # ---- FM pieces ----
                    s16 = sp.tile([P, FEAT], BF16, tag="s16")
                    hc_dT = bass.AP(hcomp.tensor, hcomp[:].offset,
                                    [hcomp[:].ap[0], (1, FEAT), (FEAT, NF)])
                    with nc.allow_low_precision(reason="bf16 FM field sums"):
                        nc.vector.tensor_reduce(out=s16[:], in_=hc_dT,
                                                axis=mybir.AxisListType.X,
                                                op=ALU.add)
                    sq_scr = sp.tile([P, NF * FEAT], BF16, tag="sq_scr")
                    sqs = sp.tile([P, 2], F32, tag="sqs")
                    nc.scalar.activation(out=sq_scr[:], in_=hcomp[:],
                                         func=AF.Square, accum_out=sqs[:, 0:1])
                    s16sq = sp.tile([P, FEAT], BF16, tag="s16sq")
                    nc.scalar.activation(out=s16sq[:], in_=s16[:],
                                         func=AF.Square, accum_out=sqs[:, 1:2])
                    nc.vector.tensor_tensor(out=fm2sb[:, t:t + 1],
                                            in0=sqs[:, 1:2], in1=sqs[:, 0:1],
                                            op=ALU.subtract)
                    # fm1 sparse: sum of col 16 over 20 fields
                    f1v = bass.AP(hsp.tensor, hsp[:, 0, FEAT:FEAT + 1].offset,
                                  [hsp[:].ap[0], (E, NS)])
                    nc.vector.tensor_reduce(out=fmA[:, t:t + 1], in_=f1v,
                                            axis=mybir.AxisListType.X,
                                            op=ALU.add)
                    # ---- transposes to feature-major ----
                    pt = ps_t.tile([P, NK, P], BF16, tag="pt")
                    for k in range(NK):
                        nf = CB[k + 1] - CB[k]
                        nc.tensor.transpose(out=pt[0:nf, k, 0:P],
                                            in_=hcomp[:, CB[k]:CB[k + 1]],
                                            identity=ident[:])
                    hTv = bass.AP(hT.tensor, hT[:, 0, t * P:(t + 1) * P].offset,
                                  [hT[:].ap[0], (BC, NK), (1, P)])
                    if t % 2 == 0:
                        nc.scalar.activation(out=hTv, in_=pt[:], func=AF.Copy)
                    else:
                        nc.vector.tensor_copy(out=hTv, in_=pt[:])

                # finalize FM: fm2sb = 0.5*fm2sb + fmA + fmB + const
                nc.vector.tensor_scalar(out=fm2sb[:], in0=fm2sb[:],
                                        scalar1=0.5, scalar2=None,
                                        op0=ALU.mult)
                nc.vector.tensor_tensor(out=fm2sb[:], in0=fm2sb[:],
                                        in1=fmA[:], op=ALU.add)
                c3 = bass.AP(vec2_t.tensor, vec2_t[:, 3:4].offset,
                             [vec2_t[:].ap[0], (0, NT)])
                nc.vector.tensor_tensor(out=fm2sb[:], in0=fm2sb[:],
                                        in1=c3, op=ALU.add)

                # ================= L1 =================
                GW = 512
                for m in range(2):
                    for g in range(4):
                        pz = ps_z.tile([P, GW], F32, tag="pz")
                        for k in range(NK):
                            nf = CB[k + 1] - CB[k]
                            nc.tensor.matmul(
                                out=pz[:],
                                lhsT=w1_t[0:nf, k, m * P:(m + 1) * P],
                                rhs=hT[0:nf, k, g * GW:(g + 1) * GW],
                                start=(k == 0), stop=(k == NK - 1))
                        nc.vector.bn_stats(out=st1[:, 4 * m + g, :], in_=pz[:])
                        if g % 2 == 0:
                            nc.scalar.activation(
                                out=z1T[m][:, g * GW:(g + 1) * GW], in_=pz[:],
                                func=AF.Copy)
                        else:
                            nc.vector.tensor_copy(
                                out=z1T[m][:, g * GW:(g + 1) * GW], in_=pz[:])

                # stats -> sums for AllReduce: (m0S, m0Q, m1S, m1Q)
                mv1 = bigp.tile([P, 2, 2], F32, tag="mv1")
                for m in range(2):
                    nc.vector.bn_aggr(out=mv1[:, m, :],
                                      in_=st1[:, 4 * m:4 * m + 4, :])
                sums1 = bigp.tile([P, 4], F32, tag="sums1")
                mvm = bass.AP(mv1.tensor, mv1[:].offset, [mv1[:].ap[0], (2, 2)])
                mvv = bass.AP(mv1.tensor, mv1[:, 0, 1:2].offset,
                              [mv1[:].ap[0], (2, 2)])
                sS = bass.AP(sums1.tensor, sums1[:].offset,
                             [sums1[:].ap[0], (2, 2)])
                sQ = bass.AP(sums1.tensor, sums1[:, 1:2].offset,
                             [sums1[:].ap[0], (2, 2)])
                nc.vector.tensor_scalar(out=sS, in0=mvm, scalar1=float(BC),
                                        scalar2=None, op0=ALU.mult)
                tmp2 = bigp.tile([P, 2], F32, tag="tmp2")
                nc.vector.tensor_tensor(out=tmp2[:], in0=mvm, in1=mvm,
                                        op=ALU.mult)
                nc.vector.tensor_tensor(out=tmp2[:], in0=tmp2[:], in1=mvv,
                                        op=ALU.add)
                nc.vector.tensor_scalar(out=sQ, in0=tmp2[:], scalar1=float(BC),
                                        scalar2=None, op0=ALU.mult)

                # ---- AllReduce #1 ----
                nc.sync.dma_start(out=cc1_in[:], in_=sums1[:])
                if skip_cc:
                    nc.sync.dma_start(out=cc1_out[:], in_=cc1_in[:])
                else:
                    nc.gpsimd.collective_compute(
                        "AllReduce", ALU.add,
                        replica_groups=[list(range(N_CORES))],
                        ins=[cc1_in[:]], outs=[cc1_out[:]])
                ar1 = bigp.tile([P, 4], F32, tag="ar1")
                nc.sync.dma_start(out=ar1[:], in_=cc1_out[:])

                # BN1 params: A = g/std, C = be - mean*A + A*b1
                a1S = bass.AP(ar1.tensor, ar1[:].offset, [ar1[:].ap[0], (2, 2)])
                a1Q = bass.AP(ar1.tensor, ar1[:, 1:2].offset,
                              [ar1[:].ap[0], (2, 2)])
                mean1 = bigp.tile([P, 2], F32, tag="mean1")
                var1 = bigp.tile([P, 2], F32, tag="var1")
                bn1A = bigp.tile([P, 2], F32, tag="bn1A")
                bn1C = bigp.tile([P, 2], F32, tag="bn1C")
                nc.vector.tensor_scalar(out=mean1[:], in0=a1S, scalar1=1.0 / B,
                                        scalar2=None, op0=ALU.mult)
                nc.vector.tensor_scalar(out=var1[:], in0=a1Q, scalar1=1.0 / B,
                                        scalar2=None, op0=ALU.mult)
                msq = bigp.tile([P, 2], F32, tag="msq")
                nc.vector.tensor_tensor(out=msq[:], in0=mean1[:], in1=mean1[:],
                                        op=ALU.mult)
                nc.vector.tensor_tensor(out=var1[:], in0=var1[:], in1=msq[:],
                                        op=ALU.subtract)
                nc.vector.tensor_scalar(out=var1[:], in0=var1[:],
                                        scalar1=BN_EPS, scalar2=None,
                                        op0=ALU.add)
                nc.scalar.activation(out=var1[:], in_=var1[:], func=AF.Sqrt)
                nc.vector.reciprocal(out=var1[:], in_=var1[:])
                nc.vector.tensor_tensor(out=bn1A[:], in0=vec1_t[:, 0:2],
                                        in1=var1[:], op=ALU.mult)
                nc.vector.tensor_tensor(out=msq[:], in0=mean1[:], in1=bn1A[:],
                                        op=ALU.mult)
                nc.vector.tensor_tensor(out=bn1C[:], in0=vec1_t[:, 2:4],
                                        in1=msq[:], op=ALU.subtract)
                nc.vector.tensor_tensor(out=msq[:], in0=bn1A[:],
                                        in1=vec1_t[:, 4:6], op=ALU.mult)
                nc.vector.tensor_tensor(out=bn1C[:], in0=bn1C[:], in1=msq[:],
                                        op=ALU.add)

                # a1 = relu(A*z1 + C) in place
                for m in range(2):
                    nc.scalar.activation(out=z1T[m][:], in_=z1T[m][:],
                                         func=AF.Relu,
                                         scale=bn1A[:, m:m + 1],
                                         bias=bn1C[:, m:m + 1])

                # ================= L2 =================
                for g in range(4):
                    pz = ps_z.tile([P, GW], F32, tag="pz")
                    for k in range(2):
                        nc.tensor.matmul(out=pz[:], lhsT=w2_t[:, k, :],
                                         rhs=z1T[k][:, g * GW:(g + 1) * GW],
                                         start=(k == 0), stop=(k == 1))
                    nc.vector.bn_stats(out=st2[:, g, :], in_=pz[:])
                    if g % 2 == 0:
                        nc.scalar.activation(out=z2T[:, g * GW:(g + 1) * GW],
                                             in_=pz[:], func=AF.Copy)
                    else:
                        nc.vector.tensor_copy(out=z2T[:, g * GW:(g + 1) * GW],
                                              in_=pz[:])

                mv2 = bigp.tile([P, 2], F32, tag="mv2")
                nc.vector.bn_aggr(out=mv2[:], in_=st2[:])
                sums2 = bigp.tile([P, 2], F32, tag="sums2")
                nc.vector.tensor_scalar(out=sums2[:, 0:1], in0=mv2[:, 0:1],
                                        scalar1=float(BC), scalar2=None,
                                        op0=ALU.mult)
                t2 = bigp.tile([P, 1], F32, tag="t2")
                nc.vector.tensor_tensor(out=t2[:], in0=mv2[:, 0:1],
                                        in1=mv2[:, 0:1], op=ALU.mult)
                nc.vector.tensor_tensor(out=t2[:], in0=t2[:], in1=mv2[:, 1:2],
                                        op=ALU.add)
                nc.vector.tensor_scalar(out=sums2[:, 1:2], in0=t2[:],
                                        scalar1=float(BC), scalar2=None,
                                        op0=ALU.mult)

                # ---- AllReduce #2 ----
                nc.sync.dma_start(out=cc2_in[:], in_=sums2[:])
                if skip_cc:
                    nc.sync.dma_start(out=cc2_out[:], in_=cc2_in[:])
                else:
                    nc.gpsimd.collective_compute(
                        "AllReduce", ALU.add,
                        replica_groups=[list(range(N_CORES))],
                        ins=[cc2_in[:]], outs=[cc2_out[:]])
                ar2 = bigp.tile([P, 2], F32, tag="ar2")
                nc.sync.dma_start(out=ar2[:], in_=cc2_out[:])

                m2 = bigp.tile([P, 1], F32, tag="m2")
                v2 = bigp.tile([P, 1], F32, tag="v2")
                bn2A = bigp.tile([P, 1], F32, tag="bn2A")
                bn2C = bigp.tile([P, 1], F32, tag="bn2C")
                nc.vector.tensor_scalar(out=m2[:], in0=ar2[:, 0:1],
                                        scalar1=1.0 / B, scalar2=None,
                                        op0=ALU.mult)
                nc.vector.tensor_scalar(out=v2[:], in0=ar2[:, 1:2],
                                        scalar1=1.0 / B, scalar2=None,
                                        op0=ALU.mult)
                ms2 = bigp.tile([P, 1], F32, tag="ms2")
                nc.vector.tensor_tensor(out=ms2[:], in0=m2[:], in1=m2[:],
                                        op=ALU.mult)
                nc.vector.tensor_tensor(out=v2[:], in0=v2[:], in1=ms2[:],
                                        op=ALU.subtract)
                nc.vector.tensor_scalar(out=v2[:], in0=v2[:], scalar1=BN_EPS,
                                        scalar2=None, op0=ALU.add)
                nc.scalar.activation(out=v2[:], in_=v2[:], func=AF.Sqrt)
                nc.vector.reciprocal(out=v2[:], in_=v2[:])
                nc.vector.tensor_tensor(out=bn2A[:], in0=vec2_t[:, 0:1],
                                        in1=v2[:], op=ALU.mult)
                nc.vector.tensor_tensor(out=ms2[:], in0=m2[:], in1=bn2A[:],
                                        op=ALU.mult)
                nc.vector.tensor_tensor(out=bn2C[:], in0=vec2_t[:, 1:2],
                                        in1=ms2[:], op=ALU.subtract)
                nc.vector.tensor_tensor(out=ms2[:], in0=bn2A[:],
                                        in1=vec2_t[:, 2:3], op=ALU.mult)
                nc.vector.tensor_tensor(out=bn2C[:], in0=bn2C[:], in1=ms2[:],
                                        op=ALU.add)

                nc.scalar.activation(out=z2T[:], in_=z2T[:], func=AF.Relu,
                                     scale=bn2A[:], bias=bn2C[:])

                # ================= L3 + output =================
                ps3 = ps_s.tile([P, NT], F32, tag="ps3")
                for t in range(NT):
                    nc.tensor.matmul(out=ps3[:, t:t + 1],
                                     lhsT=z2T[:, t * P:(t + 1) * P],
                                     rhs=w3_t[:], start=True, stop=False,
                                     skip_group_check=True)
                    nc.tensor.matmul(out=ps3[:, t:t + 1],
                                     lhsT=xdeT_t[:, t * P:(t + 1) * P],
                                     rhs=dfw_t[:], start=False, stop=True,
                                     skip_group_check=True)
                nc.vector.tensor_tensor(out=out_sb[:], in0=ps3[:],
                                        in1=fm2sb[:], op=ALU.add)
                nc.sync.dma_start(out=out[:, _rep, :], in_=out_sb[:])

    nc.compile()
    return nc


def _prep_inputs(x, emb_table, fm1_table, dense_w, dense_b, dense_fm_w,
                 dense_fm_b, W1, b1, g1, be1, W2, b2, g2, be2, W3, b3):
    tbl = np.concatenate([np.asarray(emb_table, np.float32),
                          np.asarray(fm1_table, np.float32)],
                         axis=1).astype(BF)
    tbl_f32 = np.asarray(tbl, np.float32)
    # padded sub-tables for dma_gather fields
    b35, b69 = int(OFFSETS[3]), int(OFFSETS[6])
    t35 = np.zeros((R35, EW), np.float32)
    t35[:, 0:E] = tbl_f32[b35:b35 + R35]
    t35 = t35.astype(BF)
    t69 = np.zeros((R69, EW), np.float32)
    t69[:, 0:E] = tbl_f32[b69:b69 + R69]
    t69 = t69.astype(BF)
    # W1 [528,256] -> [P, 5, 256] bf16 chunks
    W1 = np.asarray(W1, np.float32)
    w1p = np.zeros((P, NK, H1), np.float32)
    for k in range(NK):
        n = CB[k + 1] - CB[k]
        w1p[0:n, k, :] = W1[CB[k]:CB[k + 1]]
    w1p = w1p.astype(BF)
    w2p = np.asarray(W2, np.float32).reshape(2, P, H2).transpose(1, 0, 2).astype(BF)
    w3p = np.asarray(W3, np.float32).reshape(P, 1).astype(BF)
    v1 = np.zeros((P, 8), np.float32)
    v1[:, 0:2] = np.asarray(g1, np.float32).reshape(2, P).T
    v1[:, 2:4] = np.asarray(be1, np.float32).reshape(2, P).T
    v1[:, 4:6] = np.asarray(b1, np.float32).reshape(2, P).T
    v2 = np.zeros((P, 4), np.float32)
    v2[:, 0] = np.asarray(g2, np.float32)
    v2[:, 1] = np.asarray(be2, np.float32)
    v2[:, 2] = np.asarray(b2, np.float32)
    v2[:, 3] = float(np.sum(np.asarray(dense_fm_b, np.float32))) + \
        float(np.asarray(b3, np.float32).reshape(-1)[0])
    dvec = np.zeros((1, 2 * ND * FEAT + ND), np.float32)
    dvec[0, 0:ND * FEAT] = np.asarray(dense_w, np.float32).reshape(-1)
    dvec[0, ND * FEAT:2 * ND * FEAT] = np.asarray(dense_b, np.float32).reshape(-1)
    dvec[0, 2 * ND * FEAT:] = np.asarray(dense_fm_w, np.float32)
    dvec = np.repeat(dvec, P, axis=0).astype(BF)
    ident = np.eye(P, dtype=np.float32).astype(BF)

    # block-diagonal small-field table [P, NCH, 10*E]
    bdp = np.zeros((P, NCH, 10 * E), np.float32)
    for j, f in enumerate(SMALL):
        v = SPARSE_DIMS[f]
        for r in range(v):
            stack = int(SOFF[j]) + r
            c, k = stack // P, stack % P
            bdp[k, c, j * E:(j + 1) * E] = tbl_f32[int(OFFSETS[f]) + r]
    bdp = bdp.astype(BF)

    def wrap_idx(idx):
        # idx[j] -> layout[16c + j%16, j//16] replicated for 8 q7 cores
        n = idx.shape[0]
        lay = np.zeros((P, n // 16), np.int16)
        w = idx.reshape(n // 16, 16).T
        for c in range(8):
            lay[16 * c:16 * (c + 1), :] = w
        return lay

    x = np.asarray(x, np.float32)
    sp_idx = x[:, :NS].astype(np.int64)
    in_maps = []
    for cidx in range(N_CORES):
        xs = x[cidx * BC:(cidx + 1) * BC]
        si = sp_idx[cidx * BC:(cidx + 1) * BC]
        gid = (si[:, :NIND] + OFFSETS[None, :NIND]).astype(np.int32)
        gid = gid.reshape(NT, P, NIND).transpose(1, 0, 2).copy()
        xde = xs[:, NS:].astype(BF).reshape(NT, P, ND).transpose(1, 0, 2).copy()
        xdeT = np.ascontiguousarray(xs[:, NS:].T).astype(BF)
        dfw = np.asarray(dense_fm_w, np.float32).reshape(ND, 1).astype(BF)
        # dma_gather idx lists: j = (t*nf + df)*128 + p
        si_t = si.reshape(NT, P, NS)
        loc35 = (si_t[:, :, 3:6] + (OFFSETS[3:6] - b35)[None, None, :])
        i35 = wrap_idx(loc35.transpose(0, 2, 1).reshape(-1).astype(np.int16))
        loc69 = (si_t[:, :, 6:10] + (OFFSETS[6:10] - b69)[None, None, :])
        i69 = wrap_idx(loc69.transpose(0, 2, 1).reshape(-1).astype(np.int16))
        # one-hot stack [P(=chunk row), NT, NCH, P(=sample)]
        oh = np.zeros((P, NT, NCH, P), np.float32)
        srows = si[:, SMALL[0]:].astype(np.int64) + SOFF[None, :]  # [BC, 10]
        tt = np.repeat(np.arange(NT), P * 10)
        qq = np.tile(np.repeat(np.arange(P), 10), NT)
        rr = srows.reshape(-1)
        oh[rr % P, tt, rr // P, qq] = 1.0
        oh = oh.astype(BF)
        in_maps.append({
            "tbl": tbl, "t35_d": t35, "t69_d": t69, "gid_d": gid,
            "i35_d": i35, "i69_d": i69, "xde_d": xde, "xdeT_d": xdeT,
            "dfw_d": dfw, "oh_d": oh, "bd_d": bdp,
            "w1_d": w1p, "w2_d": w2p, "w3_d": w3p, "vec1_d": v1,
            "vec2_d": v2, "dvec_d": dvec, "id_d": ident,
        })
    return in_maps


def kernel(**inputs) -> np.ndarray:
    if "nc" not in _CACHE:
        _CACHE["nc"] = _build()
    nc = _CACHE["nc"]
    in_maps = _prep_inputs(**inputs)
    res = run_bass_kernel_spmd(nc, in_maps, core_ids=list(range(N_CORES)))
    y = np.empty((B, 1), np.float32)
    for c in range(N_CORES):
        o = res.results[c]["out"][:, 0, :]  # [P, NT]
        y[c * BC:(c + 1) * BC, 0] = o.T.reshape(-1)
    return y


# revision 14
# speedup vs baseline: 3.3105x; 3.3105x over previous
"""DeepFM backbone on 8 TRN2 NeuronCores — v4.

Sharding: batch 16384 -> 2048 per core; packed table replicated.
Gather strategy per core:
  - fields 0-2 (vocab 1M/100K/100K): per-(field,tile) indirect DMA (48)
  - fields 3-5 (vocab 10K): ONE dma_gather, 6144 idxs, 256B padded rows
  - fields 6-9 (vocab 1K):  ONE dma_gather, 8192 idxs, 256B padded rows
  - fields 10-19 (560 rows total): one-hot x block-diagonal matmul on PE
Per 128-sample tile: unified hsp [P,20,17] bf16 -> hcomp field-major,
FM1/FM2 pieces on DVE/ACT, 5 PE transposes -> hT, drains alternate
ACT/DVE. DNN in bf16 (f32 PSUM): L1/L2 with bn_stats/bn_aggr, two tiny
AllReduces for cross-core BN, L3 + FM add, f32 output.
"""
import sys
sys.path.insert(0, '/opt/trn_rl_repo')
import numpy as np
import ml_dtypes

import concourse.bass as bass
import concourse.bacc as bacc
import concourse.tile as tile
import concourse.mybir as mybir
from concourse.bass_utils import run_bass_kernel_spmd

# ---- problem constants (hardcoded per contract) ----
SPARSE_DIMS = [1000000, 100000, 100000, 10000, 10000, 10000, 1000, 1000, 1000,
               1000, 100, 100, 100, 100, 50, 50, 20, 20, 10, 10]
NS = 20
ND = 13
NF = NS + ND                 # 33 fields
FEAT = 16
H1, H2 = 256, 128
B = 16384
N_CORES = 8
BC = B // N_CORES            # 2048 samples per core
P = 128
NT = BC // P                 # 16 batch tiles per core
R_TOTAL = int(np.sum(SPARSE_DIMS))
OFFSETS = np.concatenate([[0], np.cumsum(SPARSE_DIMS)[:-1]]).astype(np.int64)
BN_EPS = 1e-5
F32 = mybir.dt.float32
BF16 = mybir.dt.bfloat16
I32 = mybir.dt.int32
I16 = mybir.dt.int16
AF = mybir.ActivationFunctionType
ALU = mybir.AluOpType
BF = ml_dtypes.bfloat16

E = FEAT + 1                 # 17 table cols (emb + fm1)
EW = 128                     # padded row width for dma_gather (256B)
NIND = 3                     # fields via indirect DMA
R35 = 30000                  # rows fields 3-5
R69 = 4000                   # rows fields 6-9
N35 = NT * P * 3             # 6144 idxs
N69 = NT * P * 4             # 8192 idxs
SMALL = list(range(10, 20))  # fields via one-hot matmul
SVOC = [SPARSE_DIMS[f] for f in SMALL]
SOFF = np.concatenate([[0], np.cumsum(SVOC)[:-1]]).astype(np.int64)
SROWS = int(np.sum(SVOC))    # 560
NCH = 5                      # one-hot row chunks of 128
# W1 row chunks over the 528-dim field-major input
CB = [0, 128, 256, 384, 512, 528]
NK = 5

_CACHE = {}


def _build(reps=1, skip_gather=False, skip_cc=False):
    nc = bacc.Bacc("TRN2", target_bir_lowering=False, debug=False,
                   num_devices=N_CORES)
    # ---- DRAM I/O ----
    tbl = nc.dram_tensor("tbl", [R_TOTAL, E], BF16, kind="ExternalInput")
    t35_d = nc.dram_tensor("t35_d", [R35, EW], BF16, kind="ExternalInput")
    t69_d = nc.dram_tensor("t69_d", [R69, EW], BF16, kind="ExternalInput")
    gid_d = nc.dram_tensor("gid_d", [P, NT, NIND], I32, kind="ExternalInput")
    i35_d = nc.dram_tensor("i35_d", [P, N35 // 16], I16, kind="ExternalInput")
    i69_d = nc.dram_tensor("i69_d", [P, N69 // 16], I16, kind="ExternalInput")
    xde_d = nc.dram_tensor("xde_d", [P, NT, ND], BF16, kind="ExternalInput")
    oh_d = nc.dram_tensor("oh_d", [P, NT, NCH, P], BF16, kind="ExternalInput")
    bd_d = nc.dram_tensor("bd_d", [P, NCH, 10 * E], BF16, kind="ExternalInput")
    w1_d = nc.dram_tensor("w1_d", [P, NK, H1], BF16, kind="ExternalInput")
    w2_d = nc.dram_tensor("w2_d", [P, 2, H2], BF16, kind="ExternalInput")
    w3_d = nc.dram_tensor("w3_d", [P, 1], BF16, kind="ExternalInput")
    vec1_d = nc.dram_tensor("vec1_d", [P, 8], F32, kind="ExternalInput")
    vec2_d = nc.dram_tensor("vec2_d", [P, 4], F32, kind="ExternalInput")
    dvec_d = nc.dram_tensor("dvec_d", [P, 2 * ND * FEAT + ND], BF16,
                            kind="ExternalInput")
    id_d = nc.dram_tensor("id_d", [P, P], BF16, kind="ExternalInput")
    out = nc.dram_tensor("out", [P, reps, NT], F32, kind="ExternalOutput")

    with tile.TileContext(nc) as tc:
        with tc.tile_pool(name="const", bufs=1) as cp, \
             tc.tile_pool(name="hsp", bufs=3) as hp, \
             tc.tile_pool(name="scr", bufs=3) as sp, \
             tc.tile_pool(name="big", bufs=1) as bigp, \
             tc.tile_pool(name="ps_t", bufs=2, space="PSUM") as ps_t, \
             tc.tile_pool(name="ps_g", bufs=2, space="PSUM") as ps_g, \
             tc.tile_pool(name="ps_z", bufs=2, space="PSUM") as ps_z, \
             tc.tile_pool(name="ps_s", bufs=1, space="PSUM") as ps_s, \
             tc.tile_pool(name="dram", bufs=2, space="DRAM") as dp:

            # ---- load constants ----
            gid_t = cp.tile([P, NT, NIND], I32)
            nc.sync.dma_start(out=gid_t[:], in_=gid_d[:])
            i35_t = cp.tile([P, N35 // 16], I16)
            nc.sync.dma_start(out=i35_t[:], in_=i35_d[:])
            i69_t = cp.tile([P, N69 // 16], I16)
            nc.sync.dma_start(out=i69_t[:], in_=i69_d[:])
            xde_t = cp.tile([P, NT, ND], BF16)
            nc.sync.dma_start(out=xde_t[:], in_=xde_d[:])
            oh_t = cp.tile([P, NT, NCH, P], BF16)
            nc.sync.dma_start(out=oh_t[:], in_=oh_d[:])
            bd_t = cp.tile([P, NCH, 10 * E], BF16)
            nc.sync.dma_start(out=bd_t[:], in_=bd_d[:])
            w1_t = cp.tile([P, NK, H1], BF16)
            nc.sync.dma_start(out=w1_t[:], in_=w1_d[:])
            w2_t = cp.tile([P, 2, H2], BF16)
            nc.sync.dma_start(out=w2_t[:], in_=w2_d[:])
            w3_t = cp.tile([P, 1], BF16)
            nc.sync.dma_start(out=w3_t[:], in_=w3_d[:])
            vec1_t = cp.tile([P, 8], F32)
            nc.sync.dma_start(out=vec1_t[:], in_=vec1_d[:])
            vec2_t = cp.tile([P, 4], F32)
            nc.sync.dma_start(out=vec2_t[:], in_=vec2_d[:])
            dv_t = cp.tile([P, 2 * ND * FEAT + ND], BF16)
            nc.sync.dma_start(out=dv_t[:], in_=dvec_d[:])
            ident = cp.tile([P, P], BF16)
            nc.sync.dma_start(out=ident[:], in_=id_d[:])

            for _rep in range(reps):
                hT = bigp.tile([P, NK, BC], BF16, tag="hT")
                z1T = [bigp.tile([P, BC], BF16, name=f"z1T{m}", tag=f"z1T{m}")
                       for m in range(2)]
                z2T = bigp.tile([P, BC], BF16, tag="z2T")
                g35 = bigp.tile([P, NT * 3, EW], BF16, tag="g35")
                g69 = bigp.tile([P, NT * 4, EW], BF16, tag="g69")
                fm2sb = bigp.tile([P, NT], F32, tag="fm2sb")
                fmA = bigp.tile([P, NT], F32, tag="fmA")
                fmB = bigp.tile([P, NT], F32, tag="fmB")
                st1 = bigp.tile([P, 8, 6], F32, tag="st1")
                st2 = bigp.tile([P, 4, 6], F32, tag="st2")
                out_sb = bigp.tile([P, NT], F32, tag="out_sb")

                cc1_in = dp.tile([P, 4], F32, tag="cc1i")
                cc1_out = dp.tile([P, 4], F32, tag="cc1o")
                cc2_in = dp.tile([P, 2], F32, tag="cc2i")
                cc2_out = dp.tile([P, 2], F32, tag="cc2o")

                # ---- bulk gathers for fields 3-9 ----
                if not skip_gather:
                    nc.gpsimd.dma_gather(
                        out_ap=g35[:], in_ap=t35_d[:], idxs_ap=i35_t[:],
                        num_idxs=N35, num_idxs_reg=N35, elem_size=EW,
                        single_packet=False)
                    nc.gpsimd.dma_gather(
                        out_ap=g69[:], in_ap=t69_d[:], idxs_ap=i69_t[:],
                        num_idxs=N69, num_idxs_reg=N69, elem_size=EW,
                        single_packet=False)
                else:
                    nc.vector.memset(g35[:, :, 0:E], 0.01)
                    nc.vector.memset(g69[:, :, 0:E], 0.01)

                # ================= per-tile loop =================
                for t in range(NT):
                    hsp = hp.tile([P, NS, E], BF16, tag="hsp")
                    # fields 0-2: per-partition indirect gathers
                    if skip_gather:
                        nc.vector.memset(hsp[:, 0:NIND, :], 0.01)
                    else:
                        for f in range(NIND):
                            nc.gpsimd.indirect_dma_start(
                                out=hsp[:, f, :], out_offset=None, in_=tbl[:],
                                in_offset=bass.IndirectOffsetOnAxis(
                                    ap=gid_t[:, t, f:f + 1], axis=0))
                    # fields 3-5 / 6-9 from the bulk gathers
                    s35 = bass.AP(g35.tensor, g35[:, t * 3, 0:1].offset,
                                  [g35[:].ap[0], (EW, 3), (1, E)])
                    nc.vector.tensor_copy(out=hsp[:, 3:6, :], in_=s35)
                    s69 = bass.AP(g69.tensor, g69[:, t * 4, 0:1].offset,
                                  [g69[:].ap[0], (EW, 4), (1, E)])
                    nc.vector.tensor_copy(out=hsp[:, 6:10, :], in_=s69)
                    # fields 10-19: one-hot x block-diagonal table
                    ge = ps_g.tile([P, 10 * E], F32, tag="ge")
                    for c in range(NCH):
                        cr = min(P, SROWS - c * P)
                        nc.tensor.matmul(
                            out=ge[:], lhsT=oh_t[0:cr, t, c, :],
                            rhs=bd_t[0:cr, c, :],
                            start=(c == 0), stop=(c == NCH - 1))
                    gev = bass.AP(hsp.tensor, hsp[:, 10, 0:1].offset,
                                  [hsp[:].ap[0], (1, 10 * E)])
                    nc.scalar.activation(out=gev, in_=ge[:], func=AF.Copy)

                    # hcomp: unified field-major bf16 [P, 33*16]
                    hcomp = sp.tile([P, NF * FEAT], BF16, tag="hcomp")
                    hc_sp = bass.AP(hcomp.tensor, hcomp[:].offset,
                                    [hcomp[:].ap[0], (FEAT, NS), (1, FEAT)])
                    hs_emb = bass.AP(hsp.tensor, hsp[:].offset,
                                     [hsp[:].ap[0], (E, NS), (1, FEAT)])
                    nc.vector.tensor_copy(out=hc_sp, in_=hs_emb)
                    # dense embeddings into hcomp cols 320:528
                    x3 = bass.AP(xde_t.tensor, xde_t[:, t, :].offset,
                                 [xde_t[:].ap[0], (1, ND), (0, FEAT)])
                    dw3 = bass.AP(dv_t.tensor, dv_t[:].offset,
                                  [dv_t[:].ap[0], (FEAT, ND), (1, FEAT)])
                    de = bass.AP(hcomp.tensor, hcomp[:, NS * FEAT:].offset,
                                 [hcomp[:].ap[0], (1, ND * FEAT)])
                    de2 = bass.AP(hcomp.tensor, hcomp[:, NS * FEAT:].offset,
                                  [hcomp[:].ap[0], (FEAT, ND), (1, FEAT)])
                    nc.vector.tensor_tensor(out=de2, in0=x3, in1=dw3,
                                            op=ALU.mult)
                    nc.vector.tensor_tensor(
                        out=de, in0=de, in1=dv_t[:, ND * FEAT:2 * ND * FEAT],
                        op=ALU.add)

                    # ---- FM pieces ----
                    s16 = sp.tile([P, FEAT], BF16, tag="s16")
                    hc_dT = bass.AP(hcomp.tensor, hcomp[:].offset,
                                    [hcomp[:].ap[0], (1, FEAT), (FEAT, NF)])
                    with nc.allow_low_precision(reason="bf16 FM field sums"):
                        nc.vector.tensor_reduce(out=s16[:], in_=hc_dT,
                                                axis=mybir.AxisListType.X,
                                                op=ALU.add)
                    sq_scr = sp.tile([P, NF * FEAT], BF16, tag="sq_scr")
                    sqs = sp.tile([P, 2], F32, tag="sqs")
                    nc.scalar.activation(out=sq_scr[:], in_=hcomp[:],
                                         func=AF.Square, accum_out=sqs[:, 0:1])
                    s16sq = sp.tile([P, FEAT], BF16, tag="s16sq")
                    nc.scalar.activation(out=s16sq[:], in_=s16[:],
                                         func=AF.Square, accum_out=sqs[:, 1:2])
                    nc.vector.tensor_tensor(out=fm2sb[:, t:t + 1],
                                            in0=sqs[:, 1:2], in1=sqs[:, 0:1],
                                            op=ALU.subtract)
                    # fm1 sparse: sum of col 16 over 20 fields
                    f1v = bass.AP(hsp.tensor, hsp[:, 0, FEAT:FEAT + 1].offset,
                                  [hsp[:].ap[0], (E, NS)])
                    nc.vector.tensor_reduce(out=fmA[:, t:t + 1], in_=f1v,
                                            axis=mybir.AxisListType.X,
                                            op=ALU.add)
                    # fm1 dense: sum x_de * dfw
                    dfm = sp.tile([P, ND], BF16, tag="dfm")
                    nc.vector.tensor_tensor(out=dfm[:], in0=xde_t[:, t, :],
                                            in1=dv_t[:, 2 * ND * FEAT:],
                                            op=ALU.mult)
                    nc.vector.tensor_reduce(out=fmB[:, t:t + 1], in_=dfm[:],
                                            axis=mybir.AxisListType.X,
                                            op=ALU.add)
                    # ---- transposes to feature-major ----
                    pt = ps_t.tile([P, NK, P], BF16, tag="pt")
                    for k in range(NK):
                        nf = CB[k + 1] - CB[k]
                        nc.tensor.transpose(out=pt[0:nf, k, 0:P],
                                            in_=hcomp[:, CB[k]:CB[k + 1]],
                                            identity=ident[:])
                    hTv = bass.AP(hT.tensor, hT[:, 0, t * P:(t + 1) * P].offset,
                                  [hT[:].ap[0], (BC, NK), (1, P)])
                    if t % 2 == 0:
                        nc.scalar.activation(out=hTv, in_=pt[:], func=AF.Copy)
                    else:
                        nc.vector.tensor_copy(out=hTv, in_=pt[:])

                # finalize FM: fm2sb = 0.5*fm2sb + fmA + fmB + const
                nc.vector.tensor_scalar(out=fm2sb[:], in0=fm2sb[:],
                                        scalar1=0.5, scalar2=None,
                                        op0=ALU.mult)
                nc.vector.tensor_tensor(out=fm2sb[:], in0=fm2sb[:],
                                        in1=fmA[:], op=ALU.add)
                nc.vector.tensor_tensor(out=fm2sb[:], in0=fm2sb[:],
                                        in1=fmB[:], op=ALU.add)
                c3 = bass.AP(vec2_t.tensor, vec2_t[:, 3:4].offset,
                             [vec2_t[:].ap[0], (0, NT)])
                nc.vector.tensor_tensor(out=fm2sb[:], in0=fm2sb[:],
                                        in1=c3, op=ALU.add)

                # ================= L1 =================
                GW = 512
                for m in range(2):
                    for g in range(4):
                        pz = ps_z.tile([P, GW], F32, tag="pz")
                        for k in range(NK):
                            nf = CB[k + 1] - CB[k]
                            nc.tensor.matmul(
                                out=pz[:],
                                lhsT=w1_t[0:nf, k, m * P:(m + 1) * P],
                                rhs=hT[0:nf, k, g * GW:(g + 1) * GW],
                                start=(k == 0), stop=(k == NK - 1))
                        nc.vector.bn_stats(out=st1[:, 4 * m + g, :], in_=pz[:])
                        if g % 2 == 0:
                            nc.scalar.activation(
                                out=z1T[m][:, g * GW:(g + 1) * GW], in_=pz[:],
                                func=AF.Copy)
                        else:
                            nc.vector.tensor_copy(
                                out=z1T[m][:, g * GW:(g + 1) * GW], in_=pz[:])

                # stats -> sums for AllReduce: (m0S, m0Q, m1S, m1Q)
                mv1 = bigp.tile([P, 2, 2], F32, tag="mv1")
                for m in range(2):
                    nc.vector.bn_aggr(out=mv1[:, m, :],
                                      in_=st1[:, 4 * m:4 * m + 4, :])
                sums1 = bigp.tile([P, 4], F32, tag="sums1")
                mvm = bass.AP(mv1.tensor, mv1[:].offset, [mv1[:].ap[0], (2, 2)])
                mvv = bass.AP(mv1.tensor, mv1[:, 0, 1:2].offset,
                              [mv1[:].ap[0], (2, 2)])
                sS = bass.AP(sums1.tensor, sums1[:].offset,
                             [sums1[:].ap[0], (2, 2)])
                sQ = bass.AP(sums1.tensor, sums1[:, 1:2].offset,
                             [sums1[:].ap[0], (2, 2)])
                nc.vector.tensor_scalar(out=sS, in0=mvm, scalar1=float(BC),
                                        scalar2=None, op0=ALU.mult)
                tmp2 = bigp.tile([P, 2], F32, tag="tmp2")
                nc.vector.tensor_tensor(out=tmp2[:], in0=mvm, in1=mvm,
                                        op=ALU.mult)
                nc.vector.tensor_tensor(out=tmp2[:], in0=tmp2[:], in1=mvv,
                                        op=ALU.add)
                nc.vector.tensor_scalar(out=sQ, in0=tmp2[:], scalar1=float(BC),
                                        scalar2=None, op0=ALU.mult)

                # ---- AllReduce #1 ----
                nc.sync.dma_start(out=cc1_in[:], in_=sums1[:])
                if skip_cc:
                    nc.sync.dma_start(out=cc1_out[:], in_=cc1_in[:])
                else:
                    nc.gpsimd.collective_compute(
                        "AllReduce", ALU.add,
                        replica_groups=[list(range(N_CORES))],
                        ins=[cc1_in[:]], outs=[cc1_out[:]])
                ar1 = bigp.tile([P, 4], F32, tag="ar1")
                nc.sync.dma_start(out=ar1[:], in_=cc1_out[:])

                # BN1 params: A = g/std, C = be - mean*A + A*b1
                a1S = bass.AP(ar1.tensor, ar1[:].offset, [ar1[:].ap[0], (2, 2)])
                a1Q = bass.AP(ar1.tensor, ar1[:, 1:2].offset,
                              [ar1[:].ap[0], (2, 2)])
                mean1 = bigp.tile([P, 2], F32, tag="mean1")
                var1 = bigp.tile([P, 2], F32, tag="var1")
                bn1A = bigp.tile([P, 2], F32, tag="bn1A")
                bn1C = bigp.tile([P, 2], F32, tag="bn1C")
                nc.vector.tensor_scalar(out=mean1[:], in0=a1S, scalar1=1.0 / B,
                                        scalar2=None, op0=ALU.mult)
                nc.vector.tensor_scalar(out=var1[:], in0=a1Q, scalar1=1.0 / B,
                                        scalar2=None, op0=ALU.mult)
                msq = bigp.tile([P, 2], F32, tag="msq")
                nc.vector.tensor_tensor(out=msq[:], in0=mean1[:], in1=mean1[:],
                                        op=ALU.mult)
                nc.vector.tensor_tensor(out=var1[:], in0=var1[:], in1=msq[:],
                                        op=ALU.subtract)
                nc.vector.tensor_scalar(out=var1[:], in0=var1[:],
                                        scalar1=BN_EPS, scalar2=None,
                                        op0=ALU.add)
                nc.scalar.activation(out=var1[:], in_=var1[:], func=AF.Sqrt)
                nc.vector.reciprocal(out=var1[:], in_=var1[:])
                nc.vector.tensor_tensor(out=bn1A[:], in0=vec1_t[:, 0:2],
                                        in1=var1[:], op=ALU.mult)
                nc.vector.tensor_tensor(out=msq[:], in0=mean1[:], in1=bn1A[:],
                                        op=ALU.mult)
                nc.vector.tensor_tensor(out=bn1C[:], in0=vec1_t[:, 2:4],
                                        in1=msq[:], op=ALU.subtract)
                nc.vector.tensor_tensor(out=msq[:], in0=bn1A[:],
                                        in1=vec1_t[:, 4:6], op=ALU.mult)
                nc.vector.tensor_tensor(out=bn1C[:], in0=bn1C[:], in1=msq[:],
                                        op=ALU.add)

                # a1 = relu(A*z1 + C) in place
                for m in range(2):
                    nc.scalar.activation(out=z1T[m][:], in_=z1T[m][:],
                                         func=AF.Relu,
                                         scale=bn1A[:, m:m + 1],
                                         bias=bn1C[:, m:m + 1])

                # ================= L2 =================
                for g in range(4):
                    pz = ps_z.tile([P, GW], F32, tag="pz")
                    for k in range(2):
                        nc.tensor.matmul(out=pz[:], lhsT=w2_t[:, k, :],
                                         rhs=z1T[k][:, g * GW:(g + 1) * GW],
                                         start=(k == 0), stop=(k == 1))
                    nc.vector.bn_stats(out=st2[:, g, :], in_=pz[:])
                    if g % 2 == 0:
                        nc.scalar.activation(out=z2T[:, g * GW:(g + 1) * GW],
                                             in_=pz[:], func=AF.Copy)
                    else:
                        nc.vector.tensor_copy(out=z2T[:, g * GW:(g + 1) * GW],
                                              in_=pz[:])

                mv2 = bigp.tile([P, 2], F32, tag="mv2")
                nc.vector.bn_aggr(out=mv2[:], in_=st2[:])
                sums2 = bigp.tile([P, 2], F32, tag="sums2")
                nc.vector.tensor_scalar(out=sums2[:, 0:1], in0=mv2[:, 0:1],
                                        scalar1=float(BC), scalar2=None,
                                        op0=ALU.mult)
                t2 = bigp.tile([P, 1], F32, tag="t2")
                nc.vector.tensor_tensor(out=t2[:], in0=mv2[:, 0:1],
                                        in1=mv2[:, 0:1], op=ALU.mult)
                nc.vector.tensor_tensor(out=t2[:], in0=t2[:], in1=mv2[:, 1:2],
                                        op=ALU.add)
                nc.vector.tensor_scalar(out=sums2[:, 1:2], in0=t2[:],
                                        scalar1=float(BC), scalar2=None,
                                        op0=ALU.mult)

                # ---- AllReduce #2 ----
                nc.sync.dma_start(out=cc2_in[:], in_=sums2[:])
                if skip_cc:
                    nc.sync.dma_start(out=cc2_out[:], in_=cc2_in[:])
                else:
                    nc.gpsimd.collective_compute(
                        "AllReduce", ALU.add,
                        replica_groups=[list(range(N_CORES))],
                        ins=[cc2_in[:]], outs=[cc2_out[:]])
                ar2 = bigp.tile([P, 2], F32, tag="ar2")
                nc.sync.dma_start(out=ar2[:], in_=cc2_out[:])

                m2 = bigp.tile([P, 1], F32, tag="m2")
                v2 = bigp.tile([P, 1], F32, tag="v2")
                bn2A = bigp.tile([P, 1], F32, tag="bn2A")
                bn2C = bigp.tile([P, 1], F32, tag="bn2C")
                nc.vector.tensor_scalar(out=m2[:], in0=ar2[:, 0:1],
                                        scalar1=1.0 / B, scalar2=None,
                                        op0=ALU.mult)
                nc.vector.tensor_scalar(out=v2[:], in0=ar2[:, 1:2],
                                        scalar1=1.0 / B, scalar2=None,
                                        op0=ALU.mult)
                ms2 = bigp.tile([P, 1], F32, tag="ms2")
                nc.vector.tensor_tensor(out=ms2[:], in0=m2[:], in1=m2[:],
                                        op=ALU.mult)
                nc.vector.tensor_tensor(out=v2[:], in0=v2[:], in1=ms2[:],
                                        op=ALU.subtract)
                nc.vector.tensor_scalar(out=v2[:], in0=v2[:], scalar1=BN_EPS,
                                        scalar2=None, op0=ALU.add)
                nc.scalar.activation(out=v2[:], in_=v2[:], func=AF.Sqrt)
                nc.vector.reciprocal(out=v2[:], in_=v2[:])
                nc.vector.tensor_tensor(out=bn2A[:], in0=vec2_t[:, 0:1],
                                        in1=v2[:], op=ALU.mult)
                nc.vector.tensor_tensor(out=ms2[:], in0=m2[:], in1=bn2A[:],
                                        op=ALU.mult)
                nc.vector.tensor_tensor(out=bn2C[:], in0=vec2_t[:, 1:2],
                                        in1=ms2[:], op=ALU.subtract)
                nc.vector.tensor_tensor(out=ms2[:], in0=bn2A[:],
                                        in1=vec2_t[:, 2:3], op=ALU.mult)
                nc.vector.tensor_tensor(out=bn2C[:], in0=bn2C[:], in1=ms2[:],
                                        op=ALU.add)

                nc.scalar.activation(out=z2T[:], in_=z2T[:], func=AF.Relu,
                                     scale=bn2A[:], bias=bn2C[:])

                # ================= L3 + output =================
                ps3 = ps_s.tile([P, NT], F32, tag="ps3")
                for t in range(NT):
                    nc.tensor.matmul(out=ps3[:, t:t + 1],
                                     lhsT=z2T[:, t * P:(t + 1) * P],
                                     rhs=w3_t[:], start=True, stop=True,
                                     skip_group_check=True)
                nc.vector.tensor_tensor(out=out_sb[:], in0=ps3[:],
                                        in1=fm2sb[:], op=ALU.add)
                nc.sync.dma_start(out=out[:, _rep, :], in_=out_sb[:])

    nc.compile()
    return nc


def _prep_inputs(x, emb_table, fm1_table, dense_w, dense_b, dense_fm_w,
                 dense_fm_b, W1, b1, g1, be1, W2, b2, g2, be2, W3, b3):
    tbl = np.concatenate([np.asarray(emb_table, np.float32),
                          np.asarray(fm1_table, np.float32)],
                         axis=1).astype(BF)
    tbl_f32 = np.asarray(tbl, np.float32)
    # padded sub-tables for dma_gather fields
    b35, b69 = int(OFFSETS[3]), int(OFFSETS[6])
    t35 = np.zeros((R35, EW), np.float32)
    t35[:, 0:E] = tbl_f32[b35:b35 + R35]
    t35 = t35.astype(BF)
    t69 = np.zeros((R69, EW), np.float32)
    t69[:, 0:E] = tbl_f32[b69:b69 + R69]
    t69 = t69.astype(BF)
    # W1 [528,256] -> [P, 5, 256] bf16 chunks
    W1 = np.asarray(W1, np.float32)
    w1p = np.zeros((P, NK, H1), np.float32)
    for k in range(NK):
        n = CB[k + 1] - CB[k]
        w1p[0:n, k, :] = W1[CB[k]:CB[k + 1]]
    w1p = w1p.astype(BF)
    w2p = np.asarray(W2, np.float32).reshape(2, P, H2).transpose(1, 0, 2).astype(BF)
    w3p = np.asarray(W3, np.float32).reshape(P, 1).astype(BF)
    v1 = np.zeros((P, 8), np.float32)
    v1[:, 0:2] = np.asarray(g1, np.float32).reshape(2, P).T
    v1[:, 2:4] = np.asarray(be1, np.float32).reshape(2, P).T
    v1[:, 4:6] = np.asarray(b1, np.float32).reshape(2, P).T
    v2 = np.zeros((P, 4), np.float32)
    v2[:, 0] = np.asarray(g2, np.float32)
    v2[:, 1] = np.asarray(be2, np.float32)
    v2[:, 2] = np.asarray(b2, np.float32)
    v2[:, 3] = float(np.sum(np.asarray(dense_fm_b, np.float32))) + \
        float(np.asarray(b3, np.float32).reshape(-1)[0])
    dvec = np.zeros((1, 2 * ND * FEAT + ND), np.float32)
    dvec[0, 0:ND * FEAT] = np.asarray(dense_w, np.float32).reshape(-1)
    dvec[0, ND * FEAT:2 * ND * FEAT] = np.asarray(dense_b, np.float32).reshape(-1)
    dvec[0, 2 * ND * FEAT:] = np.asarray(dense_fm_w, np.float32)
    dvec = np.repeat(dvec, P, axis=0).astype(BF)
    ident = np.eye(P, dtype=np.float32).astype(BF)

    # block-diagonal small-field table [P, NCH, 10*E]
    bdp = np.zeros((P, NCH, 10 * E), np.float32)
    for j, f in enumerate(SMALL):
        v = SPARSE_DIMS[f]
        for r in range(v):
            stack = int(SOFF[j]) + r
            c, k = stack // P, stack % P
            bdp[k, c, j * E:(j + 1) * E] = tbl_f32[int(OFFSETS[f]) + r]
    bdp = bdp.astype(BF)

    def wrap_idx(idx):
        # idx[j] -> layout[16c + j%16, j//16] replicated for 8 q7 cores
        n = idx.shape[0]
        lay = np.zeros((P, n // 16), np.int16)
        w = idx.reshape(n // 16, 16).T
        for c in range(8):
            lay[16 * c:16 * (c + 1), :] = w
        return lay

    x = np.asarray(x, np.float32)
    sp_idx = x[:, :NS].astype(np.int64)
    in_maps = []
    for cidx in range(N_CORES):
        xs = x[cidx * BC:(cidx + 1) * BC]
        si = sp_idx[cidx * BC:(cidx + 1) * BC]
        gid = (si[:, :NIND] + OFFSETS[None, :NIND]).astype(np.int32)
        gid = gid.reshape(NT, P, NIND).transpose(1, 0, 2).copy()
        xde = xs[:, NS:].astype(BF).reshape(NT, P, ND).transpose(1, 0, 2).copy()
        # dma_gather idx lists: j = (t*nf + df)*128 + p
        si_t = si.reshape(NT, P, NS)
        loc35 = (si_t[:, :, 3:6] + (OFFSETS[3:6] - b35)[None, None, :])
        i35 = wrap_idx(loc35.transpose(0, 2, 1).reshape(-1).astype(np.int16))
        loc69 = (si_t[:, :, 6:10] + (OFFSETS[6:10] - b69)[None, None, :])
        i69 = wrap_idx(loc69.transpose(0, 2, 1).reshape(-1).astype(np.int16))
        # one-hot stack [P(=chunk row), NT, NCH, P(=sample)]
        oh = np.zeros((P, NT, NCH, P), np.float32)
        srows = si[:, SMALL[0]:].astype(np.int64) + SOFF[None, :]  # [BC, 10]
        tt = np.repeat(np.arange(NT), P * 10)
        qq = np.tile(np.repeat(np.arange(P), 10), NT)
        rr = srows.reshape(-1)
        oh[rr % P, tt, rr // P, qq] = 1.0
        oh = oh.astype(BF)
        in_maps.append({
            "tbl": tbl, "t35_d": t35, "t69_d": t69, "gid_d": gid,
            "i35_d": i35, "i69_d": i69, "xde_d": xde, "oh_d": oh, "bd_d": bdp,
            "w1_d": w1p, "w2_d": w2p, "w3_d": w3p, "vec1_d": v1,
            "vec2_d": v2, "dvec_d": dvec, "id_d": ident,
        })
    return in_maps


def kernel(**inputs) -> np.ndarray:
    if "nc" not in _CACHE:
        _CACHE["nc"] = _build()
    nc = _CACHE["nc"]
    in_maps = _prep_inputs(**inputs)
    res = run_bass_kernel_spmd(nc, in_maps, core_ids=list(range(N_CORES)))
    y = np.empty((B, 1), np.float32)
    for c in range(N_CORES):
        o = res.results[c]["out"][:, 0, :]  # [P, NT]
        y[c * BC:(c + 1) * BC, 0] = o.T.reshape(-1)
    return y
